# revision 41
# baseline (speedup 1.0000x reference)
"""DigitCaps dynamic-routing kernel for 8x Trainium2 NeuronCores.

Full inputs -> batch-sharded across 8 cores (16 samples/core), W replicated.

Per-core layout:
  u_hat[(r_l,b)=128 partitions, g=256, o=16, c=10]  (bf16 in SBUF)
    where route r = g*8 + r_l   (8 routes per matmul group)
  b_ij: (128p, 256g, 10c)   (partition carries (r_l, b))

u_hat build: per group g, one PE matmul:
  lhsT = Xblk[g] (K=64=(r_l,i), M=128=(r_l,b))  block-diagonal x (host-built)
  rhs  = Wt[g]   (K=64=(r_l,i), N=160=(o,c))
  out  = psum (128=(r_l,b), 160=(o,c)) -> drain to SBUF as bf16 (2 on ACT,
  2 on DVE per 8-group sub-chunk so neither engine paces the build).

Input DMA: partition-major DRAM layout, one 9216B-contiguous run per
partition per 32-group super-chunk; supers round-robin over the sync /
scalar HWDGE rings and the gpsimd SWDGE ring for descriptor throughput.

s_j = sum_r c*u_hat: PE matmuls with constant lhsT = tile(eye(16),(8,8)):
  sums over partitions (r_l) while replicating the (16b, 160) result to all
  128 partitions; iter-1 (c==0.1 uniform) matmuls are interleaved into the
  build so they ride behind the drains.

agreement = sum_o u_hat*v: DVE broadcast multiply + tree-reduce over o.
softmax over c: free-dim innermost; squash: tiny (128,160) ops.
"""

import sys

for p in ("/opt/trn_rl_repo",):
    if p not in sys.path:
        sys.path.insert(0, p)

import numpy as np
import ml_dtypes

import concourse.bass as bass
import concourse.bacc as bacc
import concourse.mybir as mybir
import concourse.tile as tile
from concourse.bass_utils import run_bass_kernel_spmd

# Problem constants (hardcoded per contract)
B_FULL = 128
N_CORES = 8
B = B_FULL // N_CORES  # 16 samples per core
R = 2048
C = 10
O = 16
I = 8
ITERS = 3

RG = 8               # routes per matmul group
G = R // RG          # 256 groups
K = RG * I           # 64 contraction rows per group
CO = C * O           # 160
NS = 8               # DMA super-chunks (32 groups each)
CH = 64              # max groups per routing chunk (work-tile size)
# chunk starts/widths: small tail chunks shrink the end-of-iter PE tail
CHUNKS = [(0, 64), (64, 64), (128, 64), (192, 32), (224, 32)]
NCH = len(CHUNKS)
CPAD = 12            # padded capsule dim for 4B alignment of bf16 rows

F32 = mybir.dt.float32
BF16 = mybir.dt.bfloat16

_COMPILED = None  # cache (nc, names) across calls


def _host_prep(x, W):
    """Build per-core DMA-ready arrays. x: (128,2048,8) W: (2048,10,16,8)."""
    x = np.ascontiguousarray(x, dtype=np.float32)
    W = np.ascontiguousarray(W, dtype=np.float32)

    # Wt[g, r_l*8+i, o*10+c] = W[g*8+r_l, c, o, i]
    Wt = W.transpose(0, 3, 2, 1).reshape(G, RG, I, O, C).reshape(G, K, CO)
    # interleave for the build layout: per 8-group sub-chunk, group
    # g0+two*4+gp -> partitions two*64..+63, free slot gp; then pair
    # sub-chunks into 16-group DMA chunks
    Wt8 = (Wt.reshape(G // 8, 2, 4, K, CO).transpose(0, 1, 3, 2, 4)
           .reshape(G // 8, 128, 4, CO))
    Wt16 = (Wt8.reshape(G // 16, 2, 128, 4, CO).transpose(0, 2, 1, 3, 4)
            .reshape(G // 16, 128, 8, CO))

    # Bmask[(r_l,b), (r_l',b')] = 1 if b==b'  -> psum = sum over r_l,
    # replicated across all output partitions
    bmask = np.tile(np.eye(B, dtype=np.float32), (RG, RG))  # (128, 128)
    bmask_bf = bmask.astype(ml_dtypes.bfloat16)

    shards = []
    for ci in range(N_CORES):
        xs = x[ci * B : (ci + 1) * B]  # (16, 2048, 8)
        # xt[g, r_l, i, b] = xs[b, g*8+r_l, i]
        xt = xs.transpose(1, 2, 0).reshape(G, RG, I, B)
        # Block-diagonal lhsT: Xblk[g, r_l*8+i, r_l*16+b] = xt[g, r_l, i, b]
        xblk = np.zeros((G, RG, I, RG, B), dtype=np.float32)
        idx = np.arange(RG)
        xblk[:, idx, :, idx, :] = xt.transpose(1, 0, 2, 3)
        xblk = xblk.reshape(G, K, RG * B)
        xb8 = (xblk.reshape(G // 8, 2, 4, K, RG * B).transpose(0, 1, 3, 2, 4)
               .reshape(G // 8, 128, 4, RG * B))
        xb16 = (xb8.reshape(G // 16, 2, 128, 4, RG * B)
                .transpose(0, 2, 1, 3, 4).reshape(G // 16, 128, 8, RG * B))
        xwt = np.concatenate([xb16, Wt16], axis=3)  # (16, 128, 8, 288)
        # partition-major (128, 16 chunks, 8 slots, 288): multi-chunk DMA
        # supers are contiguous per-partition runs
        xpm = xwt.transpose(1, 0, 2, 3)
        shards.append(np.ascontiguousarray(xpm).astype(ml_dtypes.bfloat16))
    return shards, bmask_bf


def _build_kernel():
    nc = bacc.Bacc("TRN2", target_bir_lowering=False, debug=False,
                   num_devices=N_CORES)

    xwt_d = nc.dram_tensor("xwt", [128, 16, 8, 128 + CO], BF16,
                           kind="ExternalInput")
    bmask_d = nc.dram_tensor("bmask", [128, 128], BF16, kind="ExternalInput")
    vout_d = nc.dram_tensor("vout", [B, O, C], F32, kind="ExternalOutput")

    with tile.TileContext(nc) as tc:
        with (
            tc.tile_pool(name="persist", bufs=1) as persist,
            tc.tile_pool(name="xw", bufs=3) as xw,
            tc.tile_pool(name="work", bufs=3) as work,
            tc.tile_pool(name="psum", bufs=6, space="PSUM") as psum,
            tc.tile_pool(name="spsum", bufs=2, space="PSUM") as spsum,
        ):
            uhat = persist.tile([128, G, O, C], BF16)      # 80 KiB/part
            bij = persist.tile([128, G, C], F32)           # 10 KiB
            cbf = persist.tile([128, G, CPAD], BF16)       # 6 KiB
            bmask_b = persist.tile([128, 128], BF16)
            s_bf = persist.tile([128, O, C], BF16)
            s_sb = persist.tile([128, O, C], F32)
            sq = persist.tile([128, C], F32)
            sqb = persist.tile([128, C], BF16)
            sq2 = persist.tile([128, C], F32)
            zsum = persist.tile([128, G], F32)
            eps_t = persist.tile([128, 1], F32)
            warm_t = persist.tile([128, 128], BF16)
            nc.gpsimd.memset(eps_t[:], 1e-8)
            nc.gpsimd.memset(warm_t[:], 0.0)

            nc.scalar.dma_start(bmask_b[:], bmask_d[:])

            # PE warmup: the HAM clock gate holds PE at 1.2 GHz until it has
            # been busy ~3.4us. Dummy matmuls during the DMA-latency window
            # lift it to 2.4 GHz before the first real build matmul.
            wdum = spsum.tile([128, 3, O, C], F32, tag="sp", name="wdum")
            wap = wdum.rearrange("p t o c -> p (t o c)")[:, 0:128]
            for i in range(40):
                nc.tensor.matmul(wap, lhsT=warm_t[:],
                                 rhs=warm_t[:], start=True, stop=True)

            # ---------- Phase 1: u_hat build + interleaved iter-1 sums ----
            # Group pairs (g0+j, g0+4+j) run concurrently in PE row-groups
            # 0-63 / 64-127 (K=64 each); their outputs go to different PSUM
            # banks so the row-tiles don't serialize on the bank tracker.
            # One 9216B/partition DMA per 32-group super; within: 2 chunks
            # of 2 sub-chunks of 8 groups. Group g0+two*4+gp -> partitions
            # two*64..+63, free slot (sub*4+gp); xb = slot[..., 0:128],
            # wt = slot[..., 128:288].
            # iter 1: c == 0.1 exactly, so s1 = 0.1 * sum_r u_hat read from
            # uhat directly; group-triple matmuls issue as soon as their
            # groups are drained so PE rides just behind the DMA.
            sp = spsum.tile([128, 3, O, C], F32, tag="sp", name="sp_1")
            spt1 = spsum.tile([128, 3, O, C], F32, tag="sp", name="spt_1")
            rings = [nc.sync, nc.scalar, nc.gpsimd]
            # (first chunk, n chunks, ring): single-chunk supers up front so
            # the first matmuls start early; 3-chunk supers (13.8KB runs)
            # for descriptor throughput; round-robin over the 3 DMA rings
            supers = [(0, 1, 0), (1, 1, 1), (2, 1, 2), (3, 2, 0),
                      (5, 2, 1), (7, 3, 2), (10, 2, 0), (12, 2, 1),
                      (14, 2, 2)]
            next_m = 0
            gdone = 0
            for c0, nch, ring in supers:
                xwt_t = xw.tile([128, nch, 8, 128 + CO], BF16, tag="xwt",
                                name=f"xw{c0}")
                rings[ring].dma_start(xwt_t[:], xwt_d[:, c0 : c0 + nch])
                for ci in range(nch):
                    for sub in range(2):
                        g0 = (c0 + ci) * 16 + sub * 8
                        pst = [psum.tile([128, 2, O, C], F32, tag="ps",
                                         name=f"ps{g0}_{t}") for t in range(4)]
                        for j in range(4):
                            for two in range(2):
                                g = g0 + two * 4 + j
                                lo = (g - g0) % 8
                                sl = xwt_t[two * 64 : two * 64 + 64, ci,
                                           sub * 4 + j]
                                nc.tensor.matmul(
                                    pst[lo // 2][:, lo & 1],
                                    lhsT=sl[:, 0:128],
                                    rhs=sl[:, 128 : 128 + CO],
                                    start=True, stop=True)
                        for t in range(4):
                            eng = nc.scalar.copy if t < 2 else \
                                nc.vector.tensor_copy
                            eng(uhat[:, g0 + 2 * t : g0 + 2 * t + 2],
                                pst[t][:])
                    # issue iter-1 triples whose groups are drained; main
                    # accumulator takes m<80 so its combine overlaps the
                    # tail chunk, the rest accumulates into spt1
                    gdone += 16
                    m_avail = gdone // 3
                    for m in range(next_m, m_avail):
                        spc = sp if m < 80 else spt1
                        nc.tensor.matmul(
                            spc[:], lhsT=bmask_b[:],
                            rhs=uhat[:, 3 * m : 3 * m + 3].rearrange(
                                "p g o c -> p (g o c)"),
                            start=(m in (0, 80)), stop=(m == 79))
                        if m == 79:
                            nc.scalar.copy(s_sb[:], sp[:, 0])
                            nc.vector.tensor_add(s_sb[:], s_sb[:], sp[:, 1])
                            nc.vector.tensor_add(s_sb[:], s_sb[:], sp[:, 2])
                    next_m = m_avail
            nc.tensor.matmul(
                spt1[:, 0], lhsT=bmask_b[:],
                rhs=uhat[:, 255].rearrange("p o c -> p (o c)"),
                start=False, stop=True)
            for k in range(3):
                nc.vector.tensor_add(s_sb[:], s_sb[:], spt1[:, k])
            nc.vector.tensor_scalar_mul(s_sb[:], s_sb[:], 0.1)
            nc.vector.tensor_copy(s_bf[:], s_sb[:])

            # ---------- iterations 2..3 ----------
            # Fused per-chunk pipeline: agreement, chunk-local softmax, and
            # the c-weighted s products all run chunk-by-chunk so DVE streams
            # without inter-pass barriers; PE and ACT ride behind. prods
            # overwrites the pa tile (dead after the bij update) to halve
            # the work-pool footprint.
            # Agreement uses raw s (not v): sum_o u*v = squash_scale(b,c) *
            # sum_o u*s, so the big multiply starts right after s_combine
            # while the squash-scale chain runs concurrently; the per-(b,c)
            # scale lands in the tiny post-tree fixup instead.
            for it in range(1, ITERS):
                # main accumulator (chunks 0..NCH-2) + tail accumulator
                # (last chunk): the main 3-op combine overlaps the last
                # chunk's DVE/PE work instead of serializing after it
                sp = spsum.tile([128, 3, O, C], F32, tag="sp",
                                name=f"sp_{it}")
                spt = spsum.tile([128, 3, O, C], F32, tag="sp",
                                 name=f"spt_{it}")

                def agree_mult(ch):
                    g0, cw = CHUNKS[ch]
                    sl = slice(g0, g0 + cw)
                    pa = work.tile([128, cw, O, C], BF16, tag="prod",
                                   name=f"pa{it}_{ch}")
                    nc.vector.tensor_mul(
                        pa[:], uhat[:, sl],
                        s_bf[:].unsqueeze(1).broadcast_to((128, cw, O, C)))
                    return pa

                # software pipeline: issue chunk ch+1's big multiply while
                # chunk ch's exp sits on ACT, so DVE never waits on ACT
                pa = agree_mult(0)
                _squash_scale(nc, work, s_sb, sq, sqb, sq2, eps_t)
                for ch in range(NCH):
                    g0, cw = CHUNKS[ch]
                    sl = slice(g0, g0 + cw)
                    nc.vector.tensor_add(pa[:, :, 0:8], pa[:, :, 0:8],
                                         pa[:, :, 8:16])
                    nc.vector.tensor_add(pa[:, :, 0:4], pa[:, :, 0:4],
                                         pa[:, :, 4:8])
                    nc.vector.tensor_add(pa[:, :, 0:2], pa[:, :, 0:2],
                                         pa[:, :, 2:4])
                    nc.vector.tensor_add(pa[:, :, 0], pa[:, :, 0],
                                         pa[:, :, 1])
                    if it == 1:
                        nc.vector.tensor_mul(
                            bij[:, sl], pa[:, :, 0],
                            sqb[:].unsqueeze(1).broadcast_to((128, cw, C)))
                    else:
                        nc.vector.tensor_mul(
                            pa[:, :, 0], pa[:, :, 0],
                            sqb[:].unsqueeze(1).broadcast_to((128, cw, C)))
                        nc.vector.tensor_add(bij[:, sl],
                                             bij[:, sl], pa[:, :, 0])
                    # chunk-local softmax over c (exp on ACT)
                    cexp_t = work.tile([128, cw, C], F32, tag="cexp",
                                       name=f"ce{it}_{ch}", bufs=2)
                    nc.scalar.activation(cexp_t[:], bij[:, sl],
                                         mybir.ActivationFunctionType.Exp)
                    pa_next = agree_mult(ch + 1) if ch + 1 < NCH else None
                    nc.vector.reduce_sum(zsum[:, sl], cexp_t[:],
                                         axis=mybir.AxisListType.X)
                    nc.vector.reciprocal(zsum[:, sl], zsum[:, sl])
                    nc.vector.tensor_mul(
                        cbf[:, sl, 0:C], cexp_t[:],
                        zsum[:, sl].unsqueeze(2).broadcast_to((128, cw, C)))
                    # s products overwrite pa, in segments so PE starts on
                    # the first segment while DVE computes the next; the
                    # final chunk uses 16-group segments to cut the PE tail
                    if ch == NCH - 1:
                        segs = [(0, 16), (16, 16)]
                    else:
                        segs = [(h * 32, 32) for h in range(cw // 32)]
                    last_ch = ch == NCH - 1
                    spc = spt if last_ch else sp
                    for si, (off, sw) in enumerate(segs):
                        hsl = slice(g0 + off, g0 + off + sw)
                        nc.vector.tensor_mul(
                            pa[:, off : off + sw], uhat[:, hsl],
                            cbf[:, hsl, 0:C].unsqueeze(2)
                            .broadcast_to((128, sw, O, C)))
                        nt = sw // 3
                        rem = sw - 3 * nt
                        for j in range(nt):
                            nc.tensor.matmul(
                                spc[:], lhsT=bmask_b[:],
                                rhs=pa[:, off + 3 * j : off + 3 * j + 3]
                                .rearrange("p g o c -> p (g o c)"),
                                start=((ch == 0 or last_ch) and si == 0
                                       and j == 0),
                                stop=(last_ch is False and ch == NCH - 2
                                      and si == len(segs) - 1 and j == nt - 1
                                      and rem == 0))
                        nc.tensor.matmul(
                            spc[:, 0:rem], lhsT=bmask_b[:],
                            rhs=pa[:, off + 3 * nt : off + sw].rearrange(
                                "p g o c -> p (g o c)"),
                            start=False,
                            stop=(si == len(segs) - 1
                                  and ch in (NCH - 2, NCH - 1)))
                    if ch == NCH - 2:
                        # main combine overlaps the last chunk
                        nc.scalar.copy(s_sb[:], sp[:, 0])
                        nc.vector.tensor_add(s_sb[:], s_sb[:], sp[:, 1])
                        nc.vector.tensor_add(s_sb[:], s_sb[:], sp[:, 2])
                    pa = pa_next
                for k in range(3):
                    nc.vector.tensor_add(s_sb[:], s_sb[:], spt[:, k])
                if it < ITERS - 1:
                    nc.vector.tensor_copy(s_bf[:], s_sb[:])

            # ---------- output ----------
            _squash_scale(nc, work, s_sb, sq, sqb, sq2, eps_t)
            vfin = work.tile([128, O, C], F32, tag="vfin", bufs=1)
            nc.vector.tensor_mul(
                vfin[:], s_sb[:],
                sq[:].unsqueeze(1).broadcast_to((128, O, C)))
            nc.sync.dma_start(vout_d[:], vfin[0:B])

    nc.compile()
    return nc


def _squash_scale(nc, work, s_sb, sq, sqb, sq2, eps_t):
    """squash scale = (|s|^2/(1+|s|^2)) / sqrt(|s|^2 + 1e-8), per (b, c).

    Leaves the scale in `sq` (f32) and `sqb` (bf16); v = s * scale.
    s_sb layout (B, O, C).
    """
    P = s_sb.shape[0]
    ssq = work.tile([P, O, C], F32, tag="ssq", bufs=1)
    nc.vector.tensor_mul(ssq[:], s_sb[:], s_sb[:])
    nc.vector.reduce_sum(sq[:], ssq[:].rearrange("p o c -> p c o"),
                         axis=mybir.AxisListType.X)
    # sq2 = (1+n)*sqrt(n+1e-8);  sq = n / sq2
    nc.scalar.activation(sq2[:], sq[:], mybir.ActivationFunctionType.Sqrt,
                         bias=eps_t[0:P])
    nc.vector.scalar_tensor_tensor(
        sq2[:], sq[:], 1.0, sq2[:],
        op0=mybir.AluOpType.add, op1=mybir.AluOpType.mult)
    nc.vector.reciprocal(sq2[:], sq2[:])
    nc.vector.tensor_mul(sq[:], sq[:], sq2[:])
    nc.vector.tensor_copy(sqb[:], sq[:])


def kernel(x, W):
    global _COMPILED
    xshards, bmask = _host_prep(x, W)
    if _COMPILED is None:
        _COMPILED = _build_kernel()
    nc = _COMPILED
    in_maps = [
        {"xwt": xs, "bmask": bmask} for xs in xshards
    ]
    res = run_bass_kernel_spmd(nc, in_maps, list(range(N_CORES)))
    outs = []
    for ci in range(N_CORES):
        v = res.results[ci]["vout"]  # (16, O, C)
        outs.append(v.transpose(0, 2, 1))  # -> (16, C, O)
    return np.ascontiguousarray(np.concatenate(outs, axis=0), dtype=np.float32)


# revision 43
# speedup vs baseline: 1.0236x; 1.0236x over previous
"""DigitCaps dynamic-routing kernel for 8x Trainium2 NeuronCores.

Full inputs -> batch-sharded across 8 cores (16 samples/core), W replicated.

Per-core layout:
  u_hat[(r_l,b)=128 partitions, g=256, o=16, c=10]  (bf16 in SBUF)
    where route r = g*8 + r_l   (8 routes per matmul group)
  b_ij: (128p, 256g, 10c)   (partition carries (r_l, b))

u_hat build: per group g, one PE matmul:
  lhsT = Xblk[g] (K=64=(r_l,i), M=128=(r_l,b))  block-diagonal x (host-built)
  rhs  = Wt[g]   (K=64=(r_l,i), N=160=(o,c))
  out  = psum (128=(r_l,b), 160=(o,c)) -> drain to SBUF as bf16 (2 on ACT,
  2 on DVE per 8-group sub-chunk so neither engine paces the build).

Input DMA: partition-major DRAM layout, one 9216B-contiguous run per
partition per 32-group super-chunk; supers round-robin over the sync /
scalar HWDGE rings and the gpsimd SWDGE ring for descriptor throughput.

s_j = sum_r c*u_hat: PE matmuls with constant lhsT = tile(eye(16),(8,8)):
  sums over partitions (r_l) while replicating the (16b, 160) result to all
  128 partitions; iter-1 (c==0.1 uniform) matmuls are interleaved into the
  build so they ride behind the drains.

agreement = sum_o u_hat*v: DVE broadcast multiply + tree-reduce over o.
softmax over c: free-dim innermost; squash: tiny (128,160) ops.
"""

import sys

for p in ("/opt/trn_rl_repo",):
    if p not in sys.path:
        sys.path.insert(0, p)

import numpy as np
import ml_dtypes

import concourse.bass as bass
import concourse.bacc as bacc
import concourse.mybir as mybir
import concourse.tile as tile
from concourse.bass_utils import run_bass_kernel_spmd

# Problem constants (hardcoded per contract)
B_FULL = 128
N_CORES = 8
B = B_FULL // N_CORES  # 16 samples per core
R = 2048
C = 10
O = 16
I = 8
ITERS = 3

RG = 8               # routes per matmul group
G = R // RG          # 256 groups
K = RG * I           # 64 contraction rows per group
CO = C * O           # 160
NS = 8               # DMA super-chunks (32 groups each)
CH = 64              # max groups per routing chunk (work-tile size)
# chunk starts/widths: small tail chunks shrink the end-of-iter PE tail
CHUNKS = [(0, 64), (64, 64), (128, 64), (192, 32), (224, 32)]
NCH = len(CHUNKS)
CPAD = 12            # padded capsule dim for 4B alignment of bf16 rows

F32 = mybir.dt.float32
BF16 = mybir.dt.bfloat16

_COMPILED = None  # cache (nc, names) across calls


def _host_prep(x, W):
    """Build per-core DMA-ready arrays. x: (128,2048,8) W: (2048,10,16,8)."""
    x = np.ascontiguousarray(x, dtype=np.float32)
    W = np.ascontiguousarray(W, dtype=np.float32)

    # Wt[g, r_l*8+i, o*10+c] = W[g*8+r_l, c, o, i]
    Wt = W.transpose(0, 3, 2, 1).reshape(G, RG, I, O, C).reshape(G, K, CO)
    # interleave for the build layout: per 8-group sub-chunk, group
    # g0+two*4+gp -> partitions two*64..+63, free slot gp; then pair
    # sub-chunks into 16-group DMA chunks
    Wt8 = (Wt.reshape(G // 8, 2, 4, K, CO).transpose(0, 1, 3, 2, 4)
           .reshape(G // 8, 128, 4, CO))
    Wt16 = (Wt8.reshape(G // 16, 2, 128, 4, CO).transpose(0, 2, 1, 3, 4)
            .reshape(G // 16, 128, 8, CO))

    # Bmask[(r_l,b), (r_l',b')] = 1 if b==b'  -> psum = sum over r_l,
    # replicated across all output partitions
    bmask = np.tile(np.eye(B, dtype=np.float32), (RG, RG))  # (128, 128)
    bmask_bf = bmask.astype(ml_dtypes.bfloat16)

    shards = []
    for ci in range(N_CORES):
        xs = x[ci * B : (ci + 1) * B]  # (16, 2048, 8)
        # xt[g, r_l, i, b] = xs[b, g*8+r_l, i]
        xt = xs.transpose(1, 2, 0).reshape(G, RG, I, B)
        # Block-diagonal lhsT: Xblk[g, r_l*8+i, r_l*16+b] = xt[g, r_l, i, b]
        xblk = np.zeros((G, RG, I, RG, B), dtype=np.float32)
        idx = np.arange(RG)
        xblk[:, idx, :, idx, :] = xt.transpose(1, 0, 2, 3)
        xblk = xblk.reshape(G, K, RG * B)
        xb8 = (xblk.reshape(G // 8, 2, 4, K, RG * B).transpose(0, 1, 3, 2, 4)
               .reshape(G // 8, 128, 4, RG * B))
        xb16 = (xb8.reshape(G // 16, 2, 128, 4, RG * B)
                .transpose(0, 2, 1, 3, 4).reshape(G // 16, 128, 8, RG * B))
        xwt = np.concatenate([xb16, Wt16], axis=3)  # (16, 128, 8, 288)
        # partition-major (128, 16 chunks, 8 slots, 288): multi-chunk DMA
        # supers are contiguous per-partition runs
        xpm = xwt.transpose(1, 0, 2, 3)
        shards.append(np.ascontiguousarray(xpm).astype(ml_dtypes.bfloat16))
    return shards, bmask_bf


def _build_kernel():
    nc = bacc.Bacc("TRN2", target_bir_lowering=False, debug=False,
                   num_devices=N_CORES)

    xwt_d = nc.dram_tensor("xwt", [128, 16, 8, 128 + CO], BF16,
                           kind="ExternalInput")
    bmask_d = nc.dram_tensor("bmask", [128, 128], BF16, kind="ExternalInput")
    vout_d = nc.dram_tensor("vout", [B, O, C], F32, kind="ExternalOutput")

    with tile.TileContext(nc) as tc:
        with (
            tc.tile_pool(name="persist", bufs=1) as persist,
            tc.tile_pool(name="xw", bufs=3) as xw,
            tc.tile_pool(name="work", bufs=3) as work,
            tc.tile_pool(name="psum", bufs=6, space="PSUM") as psum,
            tc.tile_pool(name="spsum", bufs=2, space="PSUM") as spsum,
        ):
            uhat = persist.tile([128, G, O, C], BF16)      # 80 KiB/part
            bij = persist.tile([128, G, C], F32)           # 10 KiB
            cbf = persist.tile([128, G, CPAD], BF16)       # 6 KiB
            bmask_b = persist.tile([128, 128], BF16)
            s_bf = persist.tile([128, O, C], BF16)
            s_sb = persist.tile([128, O, C], F32)
            sq = persist.tile([128, C], F32)
            sqb = persist.tile([128, C], BF16)
            sq2 = persist.tile([128, C], F32)
            zsum = persist.tile([128, G], F32)
            eps_t = persist.tile([128, 1], F32)
            warm_t = persist.tile([128, 128], BF16)
            nc.gpsimd.memset(eps_t[:], 1e-8)
            nc.gpsimd.memset(warm_t[:], 0.0)

            nc.scalar.dma_start(bmask_b[:], bmask_d[:])

            # PE warmup: the HAM clock gate holds PE at 1.2 GHz until it has
            # been busy ~3.4us. Dummy matmuls during the DMA-latency window
            # lift it to 2.4 GHz before the first real build matmul.
            wdum = spsum.tile([128, 3, O, C], F32, tag="sp", name="wdum")
            wap = wdum.rearrange("p t o c -> p (t o c)")[:, 0:128]
            for i in range(40):
                nc.tensor.matmul(wap, lhsT=warm_t[:],
                                 rhs=warm_t[:], start=True, stop=True)

            # ---------- Phase 1: u_hat build + interleaved iter-1 sums ----
            # Group pairs (g0+j, g0+4+j) run concurrently in PE row-groups
            # 0-63 / 64-127 (K=64 each); their outputs go to different PSUM
            # banks so the row-tiles don't serialize on the bank tracker.
            # One 9216B/partition DMA per 32-group super; within: 2 chunks
            # of 2 sub-chunks of 8 groups. Group g0+two*4+gp -> partitions
            # two*64..+63, free slot (sub*4+gp); xb = slot[..., 0:128],
            # wt = slot[..., 128:288].
            # iter 1: c == 0.1 exactly, so s1 = 0.1 * sum_r u_hat read from
            # uhat directly; group-triple matmuls issue as soon as their
            # groups are drained so PE rides just behind the DMA.
            sp = spsum.tile([128, 3, O, C], F32, tag="sp", name="sp_1")
            rings = [nc.sync, nc.scalar, nc.gpsimd]
            # (first chunk, n chunks, ring): single-chunk supers up front so
            # the first matmuls start early; 3-chunk supers (13.8KB runs)
            # for descriptor throughput; round-robin over the 3 DMA rings
            supers = [(0, 1, 0), (1, 1, 1), (2, 1, 2), (3, 2, 0),
                      (5, 2, 1), (7, 3, 2), (10, 2, 0), (12, 2, 1),
                      (14, 2, 2)]
            next_m = 0
            gdone = 0
            for c0, nch, ring in supers:
                xwt_t = xw.tile([128, nch, 8, 128 + CO], BF16, tag="xwt",
                                name=f"xw{c0}")
                rings[ring].dma_start(xwt_t[:], xwt_d[:, c0 : c0 + nch])
                for ci in range(nch):
                    for sub in range(2):
                        g0 = (c0 + ci) * 16 + sub * 8
                        pst = [psum.tile([128, 2, O, C], F32, tag="ps",
                                         name=f"ps{g0}_{t}") for t in range(4)]
                        for j in range(4):
                            for two in range(2):
                                g = g0 + two * 4 + j
                                lo = (g - g0) % 8
                                sl = xwt_t[two * 64 : two * 64 + 64, ci,
                                           sub * 4 + j]
                                nc.tensor.matmul(
                                    pst[lo // 2][:, lo & 1],
                                    lhsT=sl[:, 0:128],
                                    rhs=sl[:, 128 : 128 + CO],
                                    start=True, stop=True)
                        for t in range(4):
                            eng = nc.scalar.copy if t < 2 else \
                                nc.vector.tensor_copy
                            eng(uhat[:, g0 + 2 * t : g0 + 2 * t + 2],
                                pst[t][:])
                # issue iter-1 triples whose groups are all drained
                gdone += nch * 16
                m_avail = gdone // 3
                for m in range(next_m, m_avail):
                    nc.tensor.matmul(
                        sp[:], lhsT=bmask_b[:],
                        rhs=uhat[:, 3 * m : 3 * m + 3].rearrange(
                            "p g o c -> p (g o c)"),
                        start=(m == 0), stop=False)
                next_m = m_avail
            nc.tensor.matmul(
                sp[:, 0], lhsT=bmask_b[:],
                rhs=uhat[:, 255].rearrange("p o c -> p (o c)"),
                start=False, stop=True)
            nc.scalar.copy(s_sb[:], sp[:, 0])
            nc.vector.tensor_add(s_sb[:], s_sb[:], sp[:, 1])
            nc.vector.tensor_add(s_sb[:], s_sb[:], sp[:, 2])
            nc.vector.tensor_scalar_mul(s_sb[:], s_sb[:], 0.1)
            nc.vector.tensor_copy(s_bf[:], s_sb[:])

            # ---------- iterations 2..3 ----------
            # Fused per-chunk pipeline: agreement, chunk-local softmax, and
            # the c-weighted s products all run chunk-by-chunk so DVE streams
            # without inter-pass barriers; PE and ACT ride behind. prods
            # overwrites the pa tile (dead after the bij update) to halve
            # the work-pool footprint.
            # Agreement uses raw s (not v): sum_o u*v = squash_scale(b,c) *
            # sum_o u*s, so the big multiply starts right after s_combine
            # while the squash-scale chain runs concurrently; the per-(b,c)
            # scale lands in the tiny post-tree fixup instead.
            for it in range(1, ITERS):
                # main accumulator (chunks 0..NCH-2) + tail accumulator
                # (last chunk): the main 3-op combine overlaps the last
                # chunk's DVE/PE work instead of serializing after it
                sp = spsum.tile([128, 3, O, C], F32, tag="sp",
                                name=f"sp_{it}")
                spt = spsum.tile([128, 3, O, C], F32, tag="sp",
                                 name=f"spt_{it}")

                def agree_mult(ch):
                    g0, cw = CHUNKS[ch]
                    sl = slice(g0, g0 + cw)
                    pa = work.tile([128, cw, O, C], BF16, tag="prod",
                                   name=f"pa{it}_{ch}")
                    nc.vector.tensor_mul(
                        pa[:], uhat[:, sl],
                        s_bf[:].unsqueeze(1).broadcast_to((128, cw, O, C)))
                    return pa

                # software pipeline: issue chunk ch+1's big multiply while
                # chunk ch's exp sits on ACT, so DVE never waits on ACT
                pa = agree_mult(0)
                _squash_scale(nc, work, s_sb, sq, sqb, sq2, eps_t)
                for ch in range(NCH):
                    g0, cw = CHUNKS[ch]
                    sl = slice(g0, g0 + cw)
                    nc.vector.tensor_add(pa[:, :, 0:8], pa[:, :, 0:8],
                                         pa[:, :, 8:16])
                    nc.vector.tensor_add(pa[:, :, 0:4], pa[:, :, 0:4],
                                         pa[:, :, 4:8])
                    nc.vector.tensor_add(pa[:, :, 0:2], pa[:, :, 0:2],
                                         pa[:, :, 2:4])
                    nc.vector.tensor_add(pa[:, :, 0], pa[:, :, 0],
                                         pa[:, :, 1])
                    if it == 1:
                        nc.vector.tensor_mul(
                            bij[:, sl], pa[:, :, 0],
                            sqb[:].unsqueeze(1).broadcast_to((128, cw, C)))
                    else:
                        nc.vector.tensor_mul(
                            pa[:, :, 0], pa[:, :, 0],
                            sqb[:].unsqueeze(1).broadcast_to((128, cw, C)))
                        nc.vector.tensor_add(bij[:, sl],
                                             bij[:, sl], pa[:, :, 0])
                    # chunk-local softmax over c (exp on ACT)
                    cexp_t = work.tile([128, cw, C], F32, tag="cexp",
                                       name=f"ce{it}_{ch}", bufs=2)
                    nc.scalar.activation(cexp_t[:], bij[:, sl],
                                         mybir.ActivationFunctionType.Exp)
                    pa_next = agree_mult(ch + 1) if ch + 1 < NCH else None
                    nc.vector.reduce_sum(zsum[:, sl], cexp_t[:],
                                         axis=mybir.AxisListType.X)
                    nc.vector.reciprocal(zsum[:, sl], zsum[:, sl])
                    nc.vector.tensor_mul(
                        cbf[:, sl, 0:C], cexp_t[:],
                        zsum[:, sl].unsqueeze(2).broadcast_to((128, cw, C)))
                    # s products overwrite pa, in segments so PE starts on
                    # the first segment while DVE computes the next; the
                    # final chunk uses 16-group segments to cut the PE tail
                    if ch == NCH - 1:
                        segs = [(0, 16), (16, 16)]
                    else:
                        segs = [(h * 32, 32) for h in range(cw // 32)]
                    last_ch = ch == NCH - 1
                    spc = spt if last_ch else sp
                    for si, (off, sw) in enumerate(segs):
                        hsl = slice(g0 + off, g0 + off + sw)
                        nc.vector.tensor_mul(
                            pa[:, off : off + sw], uhat[:, hsl],
                            cbf[:, hsl, 0:C].unsqueeze(2)
                            .broadcast_to((128, sw, O, C)))
                        nt = sw // 3
                        rem = sw - 3 * nt
                        for j in range(nt):
                            nc.tensor.matmul(
                                spc[:], lhsT=bmask_b[:],
                                rhs=pa[:, off + 3 * j : off + 3 * j + 3]
                                .rearrange("p g o c -> p (g o c)"),
                                start=((ch == 0 or last_ch) and si == 0
                                       and j == 0),
                                stop=(last_ch is False and ch == NCH - 2
                                      and si == len(segs) - 1 and j == nt - 1
                                      and rem == 0))
                        nc.tensor.matmul(
                            spc[:, 0:rem], lhsT=bmask_b[:],
                            rhs=pa[:, off + 3 * nt : off + sw].rearrange(
                                "p g o c -> p (g o c)"),
                            start=False,
                            stop=(si == len(segs) - 1
                                  and ch in (NCH - 2, NCH - 1)))
                    if ch == NCH - 2:
                        # main combine overlaps the last chunk
                        nc.scalar.copy(s_sb[:], sp[:, 0])
                        nc.vector.tensor_add(s_sb[:], s_sb[:], sp[:, 1])
                        nc.vector.tensor_add(s_sb[:], s_sb[:], sp[:, 2])
                    pa = pa_next
                for k in range(3):
                    nc.vector.tensor_add(s_sb[:], s_sb[:], spt[:, k])
                if it < ITERS - 1:
                    nc.vector.tensor_copy(s_bf[:], s_sb[:])

            # ---------- output ----------
            _squash_scale(nc, work, s_sb, sq, sqb, sq2, eps_t)
            vfin = work.tile([128, O, C], F32, tag="vfin", bufs=1)
            nc.vector.tensor_mul(
                vfin[:], s_sb[:],
                sq[:].unsqueeze(1).broadcast_to((128, O, C)))
            nc.sync.dma_start(vout_d[:], vfin[0:B])

    nc.compile()
    return nc


def _squash_scale(nc, work, s_sb, sq, sqb, sq2, eps_t):
    """squash scale = (|s|^2/(1+|s|^2)) / sqrt(|s|^2 + 1e-8), per (b, c).

    Leaves the scale in `sq` (f32) and `sqb` (bf16); v = s * scale.
    s_sb layout (B, O, C).
    """
    P = s_sb.shape[0]
    ssq = work.tile([P, O, C], F32, tag="ssq", bufs=1)
    nc.vector.tensor_mul(ssq[:], s_sb[:], s_sb[:])
    nc.vector.reduce_sum(sq[:], ssq[:].rearrange("p o c -> p c o"),
                         axis=mybir.AxisListType.X)
    # sq2 = (1+n)*sqrt(n+1e-8);  sq = n / sq2
    nc.scalar.activation(sq2[:], sq[:], mybir.ActivationFunctionType.Sqrt,
                         bias=eps_t[0:P])
    nc.vector.scalar_tensor_tensor(
        sq2[:], sq[:], 1.0, sq2[:],
        op0=mybir.AluOpType.add, op1=mybir.AluOpType.mult)
    nc.vector.reciprocal(sq2[:], sq2[:])
    nc.vector.tensor_mul(sq[:], sq[:], sq2[:])
    nc.vector.tensor_copy(sqb[:], sq[:])


def kernel(x, W):
    global _COMPILED
    xshards, bmask = _host_prep(x, W)
    if _COMPILED is None:
        _COMPILED = _build_kernel()
    nc = _COMPILED
    in_maps = [
        {"xwt": xs, "bmask": bmask} for xs in xshards
    ]
    res = run_bass_kernel_spmd(nc, in_maps, list(range(N_CORES)))
    outs = []
    for ci in range(N_CORES):
        v = res.results[ci]["vout"]  # (16, O, C)
        outs.append(v.transpose(0, 2, 1))  # -> (16, C, O)
    return np.ascontiguousarray(np.concatenate(outs, axis=0), dtype=np.float32)
